# revision 1
# baseline (speedup 1.0000x reference)
"""Cross-attention (GQA + RoPE) Trainium2 Bass kernel.

Sharding: 8 cores = 4 batches x 2 head-groups.
  core i -> batch b = i // 2, head-group g = i % 2
  Each core computes 8 query heads / 2 kv heads of one batch and a
  row-parallel partial of the output projection; the host sums the two
  partials per batch.

Per-core layout (all "T" tensors have head_dim / feature on partitions):
  qT   [1024, TQ]   query^T               (host-transposed)
  kvT  [1024, TKV]  key_value^T           (host-transposed)
  wq   [1024, 512]  w_q columns of this head group, head-PERMUTED so that
                    pair-tile j holds local heads (j, j+4) -> rows (0-63, 64-127).
                    This makes the Q row base (64*(h//4)) equal the K row base
                    for every head (required: matmul lhsT/rhs partition bases
                    must match the PE row placement).
  wk   [1024, 128]  w_k columns (2 kv heads)
  wv   [1024, 128]  w_v columns
  wout [512, 1024]  w_out rows, same head permutation as wq columns
  cosF [128, TKV]   rope cos stacked [c;c;c;c]   (32 rows repeated)
  sinF [128, TKV]   rope sin stacked [-s;s;-s;s]
  maskb [128, NCH]  additive kv-mask bias per 128-chunk (0 / -30000)

Algorithm per core:
  K^T = rope(wk^T @ kvT)      resident [128, TKV]   (2 kv heads stacked)
  V   = (kvT chunks)^T @ wv   resident [128, 65*NCH] per kv head, with an
                              appended ones-column per chunk (softmax denom)
  per tq block T2, per head:
     scores^T chunk [tkv 128, tq T2] = K_c^T.T @ Q^T   (PSUM)
     e = exp(0.125*scores^T + mask_bias)               (ACT, bias per partition)
     psum_o [65, T2] += V_c_aug.T @ e                  (row 64 = sum of exp)
     attnT = psum_o[0:64] * broadcast(1/psum_o[64])    (DVE + gpsimd bcast)
  out[tq, :] partial = attnT.T @ wout                  (PSUM -> DMA)
"""

import os
from contextlib import ExitStack

import numpy as np

import concourse.bass as bass
import concourse.bacc as bacc
import concourse.mybir as mybir
import concourse.tile as tile
from concourse.bass_utils import run_bass_kernel_spmd

F32 = mybir.dt.float32
R32 = mybir.dt.float32r

D_MODEL = 1024
N_HEADS = 16
NUM_KV_HEADS = 4
D_K = 64
ROPE_BASE = 10000.0
B = 4
TQ = 2048
TKV = 2048
N_CORES = 8

NEG_BIAS = -30000.0


def build_bass(tq=TQ, tkv=TKV, t2=1024, use_f32r=True):
    """Build the single-core SPMD program (same program on all 8 cores)."""
    nc = bacc.Bacc("TRN2", target_bir_lowering=False, debug=False)
    P = 128
    NKT = tkv // 512          # kv projection tiles
    NCH = tkv // 128          # attention kv chunks
    NT2 = tq // t2            # tq blocks
    NHALF = t2 // 512         # 512-wide matmul slices per tq block
    NPAIR = 4                 # head-pair tiles per core
    DT = R32 if use_f32r else F32

    qT = nc.dram_tensor("qT", [D_MODEL, tq], DT, kind="ExternalInput").ap()
    kvT = nc.dram_tensor("kvT", [D_MODEL, tkv], DT, kind="ExternalInput").ap()
    wq = nc.dram_tensor("wq", [D_MODEL, 512], DT, kind="ExternalInput").ap()
    wk = nc.dram_tensor("wk", [D_MODEL, 128], DT, kind="ExternalInput").ap()
    wv = nc.dram_tensor("wv", [D_MODEL, 128], DT, kind="ExternalInput").ap()
    wout = nc.dram_tensor("wout", [512, D_MODEL], DT, kind="ExternalInput").ap()
    cosF = nc.dram_tensor("cosF", [P, tkv], F32, kind="ExternalInput").ap()
    sinF = nc.dram_tensor("sinF", [P, tkv], F32, kind="ExternalInput").ap()
    maskb = nc.dram_tensor("maskb", [P, NCH], F32, kind="ExternalInput").ap()
    onesc = nc.dram_tensor("onesc", [P, 64], DT, kind="ExternalInput").ap()
    out = nc.dram_tensor("out", [tq, D_MODEL], F32, kind="ExternalOutput").ap()

    with tile.TileContext(nc) as tc, ExitStack() as ctx:
        const = ctx.enter_context(tc.tile_pool(name="const", bufs=1))
        blkp = ctx.enter_context(tc.tile_pool(name="blkp", bufs=2))
        qpool = ctx.enter_context(tc.tile_pool(name="qpool", bufs=1))
        apool = ctx.enter_context(tc.tile_pool(name="apool", bufs=1))
        workp = ctx.enter_context(tc.tile_pool(name="workp", bufs=3))
        ropep = ctx.enter_context(tc.tile_pool(name="ropep", bufs=2))
        outp = ctx.enter_context(tc.tile_pool(name="outp", bufs=2))
        pp_big = ctx.enter_context(tc.tile_pool(name="pp_big", bufs=2, space="PSUM"))
        pp_acc = ctx.enter_context(tc.tile_pool(name="pp_acc", bufs=2, space="PSUM"))

        def MM(out_ap, lhsT, rhs, start, stop, chain=None):
            inst = nc.tensor.matmul(out_ap, lhsT, rhs, start=start, stop=stop)
            if chain is not None:
                tc.chain_iter_dep(chain, inst.ins)
            return inst

        def chain_dve(inst):
            tc.chain_iter_dep("dve_norm", inst.ins)
            return inst

        # ---- constants / weights -------------------------------------------------
        wq_sb = const.tile([P, 8, 512], DT)
        nc.gpsimd.dma_start(out=wq_sb, in_=wq.rearrange("(c p) f -> p c f", p=P))
        wk_sb = const.tile([P, 8, 128], DT)
        nc.gpsimd.dma_start(out=wk_sb, in_=wk.rearrange("(c p) f -> p c f", p=P))
        wv_sb = const.tile([P, 8, 128], DT)
        nc.gpsimd.dma_start(out=wv_sb, in_=wv.rearrange("(c p) f -> p c f", p=P))
        wout_sb = const.tile([P, 4, D_MODEL], DT)
        nc.gpsimd.dma_start(out=wout_sb, in_=wout.rearrange("(c p) f -> p c f", p=P))
        cos_sb = const.tile([P, tkv], F32)
        nc.gpsimd.dma_start(out=cos_sb, in_=cosF)
        sin_sb = const.tile([P, tkv], F32)
        nc.gpsimd.dma_start(out=sin_sb, in_=sinF)
        mask_sb = const.tile([P, NCH], F32)
        nc.gpsimd.dma_start(out=mask_sb, in_=maskb)

        Kt = const.tile([P, tkv], DT)
        Vt = [const.tile([P, NCH * 65], DT, name=f"Vt{i}") for i in range(2)]
        for i in range(2):
            nc.gpsimd.dma_start(
                out=Vt[i].rearrange("p (c k) -> p c k", k=65)[:, :, 64],
                in_=onesc[:, :NCH],
            )
        ones_sb = const.tile([1, 64], DT)
        nc.gpsimd.dma_start(out=ones_sb, in_=onesc[0:1, :])

        def rope_apply(dest, ps, col0, width):
            """dest[128, width] (SBUF) = rope(ps[128, width] PSUM), positions
            col0..col0+width. Rows are two stacked heads, each [x1(32); x2(32)]."""
            cs = cos_sb[:, col0 : col0 + width]
            t_cos = ropep.tile([P, t2], F32, tag="rope", name="t_cos")
            t_u = ropep.tile([P, t2], F32, tag="rope", name="t_u")
            tc_ = t_cos[:, :width]
            tu_ = t_u[:, :width]
            nc.vector.tensor_mul(tc_, ps, cs)
            for b0 in (0, 64):
                # sinF rows [b0:b0+32] = -sin, [b0+32:b0+64] = +sin
                nc.vector.tensor_mul(
                    tu_[b0 : b0 + 32, :],
                    ps[b0 + 32 : b0 + 64, :],
                    sin_sb[b0 : b0 + 32, col0 : col0 + width],
                )
                nc.vector.tensor_mul(
                    tu_[b0 + 32 : b0 + 64, :],
                    ps[b0 : b0 + 32, :],
                    sin_sb[b0 + 32 : b0 + 64, col0 : col0 + width],
                )
            nc.vector.tensor_add(dest, tc_, tu_)

        # ---- phase KV: K/V projections ------------------------------------------
        for kt in range(NKT):
            kv_blk = blkp.tile([P, 8, 512], DT, tag="blk", name="kv_blk")
            nc.gpsimd.dma_start(
                out=kv_blk,
                in_=kvT.rearrange("(c p) t -> p c t", p=P)[
                    :, :, kt * 512 : (kt + 1) * 512
                ],
            )
            ps_k = pp_big.tile([P, 512], F32, tag="big", name="ps_k")
            for d in range(8):
                MM(ps_k, wk_sb[:, d, :], kv_blk[:, d, :], d == 0, d == 7)
            rope_apply(Kt[:, kt * 512 : (kt + 1) * 512], ps_k, kt * 512, 512)
            for s in range(4):
                ps_v = pp_big.tile([P, 512], F32, tag="big", name="ps_v")
                pv = ps_v[:, 0:128]
                for d in range(8):
                    MM(
                        pv,
                        kv_blk[:, d, s * 128 : (s + 1) * 128],
                        wv_sb[:, d, :],
                        d == 0,
                        d == 7,
                    )
                c = kt * 4 + s
                nc.vector.tensor_copy(
                    out=Vt[0][:, c * 65 : c * 65 + 64], in_=pv[:, 0:64]
                )
                nc.vector.tensor_copy(
                    out=Vt[1][:, c * 65 : c * 65 + 64], in_=pv[:, 64:128]
                )

        # ---- per tq block: Q proj -> attention -> output projection -------------
        for it2 in range(NT2):
            q_blks = []
            for half in range(NHALF):
                qb = blkp.tile([P, 8, 512], DT, tag="blk", name="q_blk")
                c0 = it2 * t2 + half * 512
                nc.gpsimd.dma_start(
                    out=qb,
                    in_=qT.rearrange("(c p) t -> p c t", p=P)[:, :, c0 : c0 + 512],
                )
                q_blks.append(qb)

            Qt = []
            for j in range(NPAIR):
                ps_q = pp_big.tile([P, t2], F32, tag="big", name="ps_q")
                for half in range(NHALF):
                    for d in range(8):
                        MM(
                            ps_q[:, half * 512 : (half + 1) * 512],
                            wq_sb[:, d, j * 128 : (j + 1) * 128],
                            q_blks[half][:, d, :],
                            d == 0,
                            d == 7,
                        )
                qt = qpool.tile([P, t2], DT, tag=f"Q{j}", name=f"Qt{j}")
                rope_apply(qt, ps_q, it2 * t2, t2)
                Qt.append(qt)

            attnT = [
                apool.tile([P, t2], DT, tag=f"A{j}", name=f"attnT{j}")
                for j in range(NPAIR)
            ]

            # normalization of head h is EMITTED after head h+1's attention
            # matmuls: the broadcast matmul would otherwise head-of-line block
            # the in-order PE queue on the (slow, [1,t2]) DVE reciprocal.
            pending = []

            def flush_norm():
                if not pending:
                    return
                U, inv, j_, base_ = pending.pop(0)
                for half in range(NHALF):
                    hs = slice(half * 512, (half + 1) * 512)
                    ps_b = pp_big.tile([64, 512], F32, tag="big", name="ps_b")
                    MM(ps_b, ones_sb, inv[:, hs], True, True, chain="pe_attn")
                    chain_dve(
                        nc.vector.tensor_mul(
                            attnT[j_][base_ : base_ + 64, hs], U[0:64, hs], ps_b
                        )
                    )

            # two heads (j, j+4) interleave: while one head's exp is on the
            # Scalar engine, the PE runs the other head's matmuls back-to-back
            # (keeps the PE activity window busy -> HAM stays at K=8/8).
            for j in range(NPAIR):
                heads = [(j, 0, 0), (j + 4, 1, 64)]  # (head, kvh, base)
                ps_os = [
                    pp_acc.tile([65, t2], F32, tag="acc", name=f"ps_o{ab}")
                    for ab in range(2)
                ]
                def emit_pv(c_, exs_):
                    for ab, (_h, kvh, _base) in enumerate(heads):
                        for half in range(NHALF):
                            MM(
                                ps_os[ab][:, half * 512 : (half + 1) * 512],
                                Vt[kvh][:, c_ * 65 : c_ * 65 + 65],
                                exs_[ab][:, half * 512 : (half + 1) * 512],
                                c_ == 0,
                                c_ == NCH - 1,
                                chain="pe_attn",
                            )

                # PV lags the scores by one chunk so no PE instruction ever
                # reaches the queue head with an unresolved wait (embedded
                # stalls keep the HAM activity window cold).
                prev = None
                for c in range(NCH):
                    exs = []
                    for ab, (_h, kvh, base) in enumerate(heads):
                        ps_s = pp_big.tile([P, t2], F32, tag="big", name="ps_s")
                        for half in range(NHALF):
                            MM(
                                ps_s[:, half * 512 : (half + 1) * 512],
                                Kt[base : base + 64, c * 128 : (c + 1) * 128],
                                Qt[j][base : base + 64, half * 512 : (half + 1) * 512],
                                True,
                                True,
                                chain="pe_attn",
                            )
                        ex = workp.tile([P, t2], DT, tag="expT", name="ex", bufs=4)
                        nc.scalar.activation(
                            out=ex,
                            in_=ps_s,
                            func=mybir.ActivationFunctionType.Exp,
                            bias=mask_sb[:, c : c + 1],
                            scale=0.125,
                        )
                        exs.append(ex)
                    if prev is not None:
                        emit_pv(c - 1, prev)
                    prev = exs
                emit_pv(NCH - 1, prev)
                # flush the previous pair first: its bcast matmul runs now
                # (reciprocal long done), and its muls free ps_b slots early.
                while pending:
                    flush_norm()
                # both accumulator copies BEFORE the slow reciprocals: the
                # in-order DVE must release both PSUM slots promptly.
                Us = []
                for ab in range(2):
                    U = workp.tile([65, t2], F32, tag="unorm", name="U", bufs=4)
                    chain_dve(nc.vector.tensor_copy(out=U, in_=ps_os[ab]))
                    Us.append(U)
                for ab, (_h, kvh, base) in enumerate(heads):
                    U = Us[ab]
                    inv = workp.tile([1, t2], DT, tag="inv", name="inv", bufs=3)
                    with nc.allow_low_precision("f32r denom feeds bcast matmul"):
                        chain_dve(nc.vector.reciprocal(out=inv, in_=U[64:65, :]))
                    pending.append((U, inv, j, base))
            while pending:
                flush_norm()

            for s in range(t2 // 128):
                ob = outp.tile([P, D_MODEL], F32, tag="ob", name="ob")
                for n in range(2):
                    ps_f = pp_big.tile([P, 512], F32, tag="big", name="ps_f")
                    for p_ in range(NPAIR):
                        MM(
                            ps_f,
                            attnT[p_][:, s * 128 : (s + 1) * 128],
                            wout_sb[:, p_, n * 512 : (n + 1) * 512],
                            p_ == 0,
                            p_ == NPAIR - 1,
                        )
                    nc.vector.tensor_copy(
                        out=ob[:, n * 512 : (n + 1) * 512], in_=ps_f
                    )
                r0 = it2 * t2 + s * 128
                nc.sync.dma_start(out=out[r0 : r0 + 128, :], in_=ob)

    nc.compile()
    return nc


# ---------------------------------------------------------------------------
# host-side sharding / prep
# ---------------------------------------------------------------------------

_HEAD_PERM = [0, 4, 1, 5, 2, 6, 3, 7]  # local head order inside pair tiles


def _rope_tables(tkv):
    theta = ROPE_BASE ** (-np.arange(0, D_K, 2, dtype=np.float32) / D_K)  # [32]
    pos = np.arange(tkv, dtype=np.float32)[:, None]  # [tkv,1]
    ang = pos * theta[None, :]  # [tkv,32]
    c = np.cos(ang).T.astype(np.float32)  # [32, tkv]
    s = np.sin(ang).T.astype(np.float32)
    cosF = np.concatenate([c, c, c, c], axis=0)
    sinF = np.concatenate([-s, s, -s, s], axis=0)
    return np.ascontiguousarray(cosF), np.ascontiguousarray(sinF)


def make_in_maps(query, key_value, kv_mask, w_q, w_k, w_v, w_out, tq=TQ, tkv=TKV):
    nb = query.shape[0]
    cosF, sinF = _rope_tables(max(tq, tkv))
    cosF = cosF[:, :tkv] if cosF.shape[1] != tkv else cosF
    sinF = sinF[:, :tkv] if sinF.shape[1] != tkv else sinF
    cosQ = cosF  # same tables sliced by column inside the kernel
    del cosQ
    in_maps = []
    col_perm = np.concatenate(
        [np.arange(h * D_K, (h + 1) * D_K) for h in _HEAD_PERM]
    )
    for core in range(2 * nb):
        b = core // 2
        g = core % 2
        qTb = np.ascontiguousarray(query[b].T.astype(np.float32))
        kvTb = np.ascontiguousarray(key_value[b].T.astype(np.float32))
        wq_g = w_q[:, g * 512 : (g + 1) * 512][:, col_perm]
        wk_g = w_k[:, g * 128 : (g + 1) * 128]
        wv_g = w_v[:, g * 128 : (g + 1) * 128]
        wout_g = w_out[g * 512 : (g + 1) * 512, :][col_perm, :]
        maskb = np.where(kv_mask[b], 0.0, NEG_BIAS).astype(np.float32)
        maskb = np.ascontiguousarray(maskb.reshape(tkv // 128, 128).T)
        ones_arr = np.ones((128, 64), np.float32)
        in_maps.append(
            {
                "qT": qTb,
                "kvT": kvTb,
                "wq": np.ascontiguousarray(wq_g.astype(np.float32)),
                "wk": np.ascontiguousarray(wk_g.astype(np.float32)),
                "wv": np.ascontiguousarray(wv_g.astype(np.float32)),
                "wout": np.ascontiguousarray(wout_g.astype(np.float32)),
                "cosF": cosF,
                "sinF": sinF,
                "maskb": maskb,
                "onesc": ones_arr,
            }
        )
    return in_maps


_NC_CACHE = {}


def _get_nc(tq=TQ, tkv=TKV, t2=1024, use_f32r=True):
    key = (tq, tkv, t2, use_f32r)
    if key not in _NC_CACHE:
        _NC_CACHE[key] = build_bass(tq, tkv, t2, use_f32r)
    return _NC_CACHE[key]


def _run(inputs, trace=False):
    query = np.asarray(inputs["query"], dtype=np.float32)
    key_value = np.asarray(inputs["key_value"], dtype=np.float32)
    kv_mask = np.asarray(inputs["kv_mask"])
    w_q = np.asarray(inputs["w_q"], dtype=np.float32)
    w_k = np.asarray(inputs["w_k"], dtype=np.float32)
    w_v = np.asarray(inputs["w_v"], dtype=np.float32)
    w_out = np.asarray(inputs["w_out"], dtype=np.float32)
    nb, tq, _ = query.shape
    tkv = key_value.shape[1]

    nc = _get_nc(tq, tkv)
    in_maps = make_in_maps(query, key_value, kv_mask, w_q, w_k, w_v, w_out, tq, tkv)
    res = run_bass_kernel_spmd(
        nc, in_maps, list(range(2 * nb)), trace=trace, trace_cores=[0]
    )
    outs = [np.asarray(r["out"]) for r in res.results]
    full = np.stack([outs[2 * b] + outs[2 * b + 1] for b in range(nb)])

    query_mask = np.asarray(inputs["query_mask"])
    if not query_mask.all():
        # masked query rows: reference yields uniform attention over all kv
        for b in range(nb):
            rows = ~query_mask[b]
            if rows.any():
                V = key_value[b] @ w_v  # [tkv, 256]
                meanV = V.mean(axis=0)  # [256]
                group = N_HEADS // NUM_KV_HEADS
                feat = np.concatenate([meanV.reshape(NUM_KV_HEADS, D_K)[h // group]
                                       for h in range(N_HEADS)])
                full[b, rows, :] = feat @ w_out
    return full.astype(np.float32), res


def kernel(**inputs):
    out, _ = _run(inputs, trace=False)
    return out


def kernel_traced(**inputs):
    out, res = _run(inputs, trace=True)
    return out, res


if __name__ == "__main__":
    print("kernel.py is a library; use test.py")



# revision 19
# speedup vs baseline: 1.5894x; 1.5894x over previous
"""Cross-attention (GQA + RoPE) Trainium2 Bass kernel.

Sharding: 8 cores = 4 batches x 2 head-groups.
  core i -> batch b = i // 2, head-group g = i % 2
  Each core computes 8 query heads / 2 kv heads of one batch and a
  row-parallel partial of the output projection; the host sums the two
  partials per batch.

Key optimizations over the v1 baseline:
  * kv compaction: ~50% of kv positions are masked out; the host gathers
    valid positions (and their RoPE phase tables) and pads to a multiple
    of 128 (TKV_C).  Scores / exp / PV / KV-projection all shrink.
  * All inputs are host-pre-arranged into the exact SBUF layout
    [128, c, X] so every DMA moves large contiguous per-partition rows;
    two DMA queues run in parallel (kv-side on gpsimd, q-side on vector).
  * bf16 inputs and intermediates (PSUM stays f32).
  * reciprocal_approx_fast on a [2, t2] packed denominator pair, one
    K=2 broadcast matmul per head-pair (inv0 -> psum rows 0-63,
    inv1 -> rows 64-127).
  * Software pipelining: the next block's Q-projection pairs and the
    previous block's output-projection slices are interleaved between
    attention head-pairs, so the PE never sits in a dedicated
    projection phase; output-projection PSUM->SBUF copies run on the
    Scalar engine.
"""

import math
from contextlib import ExitStack

import numpy as np
import ml_dtypes

import concourse.bass as bass
import concourse.bacc as bacc
import concourse.mybir as mybir
import concourse.tile as tile
from concourse.bass_utils import run_bass_kernel_spmd

F32 = mybir.dt.float32
R32 = mybir.dt.float32r
BF16 = mybir.dt.bfloat16

D_MODEL = 1024
N_HEADS = 16
NUM_KV_HEADS = 4
D_K = 64
ROPE_BASE = 10000.0
TQ = 2048
N_CORES = 8

NEG_BIAS = -30000.0


def build_bass(tq=TQ, tkv_c=1152, t2=1024):
    """Build the single-core SPMD program (same program on all 8 cores)."""
    nc = bacc.Bacc("TRN2", target_bir_lowering=False, debug=False)
    P = 128
    NCH = tkv_c // 128        # attention kv chunks
    NT2 = tq // t2            # tq blocks
    NHALF = t2 // 512         # 512-wide matmul slices per tq block
    NPAIR = 4                 # head-pair tiles per core
    NSLICE = t2 // 128        # output rows per block
    ktiles = []
    c0 = 0
    while c0 < tkv_c:
        w = min(512, tkv_c - c0)
        ktiles.append((c0, w))
        c0 += w

    q_in = [
        nc.dram_tensor(f"q{i}", [P, 8, t2], BF16, kind="ExternalInput").ap()
        for i in range(NT2)
    ]
    kv_a = nc.dram_tensor("kv_a", [P, 4, tkv_c], BF16, kind="ExternalInput").ap()
    kv_b = nc.dram_tensor("kv_b", [P, 4, tkv_c], BF16, kind="ExternalInput").ap()
    wq = nc.dram_tensor("wq", [P, 8, 512], BF16, kind="ExternalInput").ap()
    wk = nc.dram_tensor("wk", [P, 8, 128], BF16, kind="ExternalInput").ap()
    wv = nc.dram_tensor("wv", [P, 8, 128], BF16, kind="ExternalInput").ap()
    wout = nc.dram_tensor("wout", [P, 4, D_MODEL], BF16, kind="ExternalInput").ap()
    cosK = nc.dram_tensor("cosK", [P, tkv_c], BF16, kind="ExternalInput").ap()
    sinK = nc.dram_tensor("sinK", [P, tkv_c], BF16, kind="ExternalInput").ap()
    cosQ = nc.dram_tensor("cosQ", [P, tq], BF16, kind="ExternalInput").ap()
    sinQ = nc.dram_tensor("sinQ", [P, tq], BF16, kind="ExternalInput").ap()
    maskb = nc.dram_tensor("maskb", [P, NCH], F32, kind="ExternalInput").ap()
    e2 = nc.dram_tensor("e2", [64, P], R32, kind="ExternalInput").ap()
    out = nc.dram_tensor("out", [tq, D_MODEL], F32, kind="ExternalOutput").ap()

    with tile.TileContext(nc) as tc, ExitStack() as ctx:
        const = ctx.enter_context(tc.tile_pool(name="const", bufs=1))
        qpool = ctx.enter_context(tc.tile_pool(name="qpool", bufs=1))
        apool = ctx.enter_context(tc.tile_pool(name="apool", bufs=1))
        workp = ctx.enter_context(tc.tile_pool(name="workp", bufs=3))
        ropep = ctx.enter_context(tc.tile_pool(name="ropep", bufs=2))
        pp_big = ctx.enter_context(tc.tile_pool(name="pp_big", bufs=2, space="PSUM"))
        pp_acc = ctx.enter_context(tc.tile_pool(name="pp_acc", bufs=2, space="PSUM"))

        def MM(out_ap, lhsT, rhs, start, stop, chain=None):
            inst = nc.tensor.matmul(out_ap, lhsT, rhs, start=start, stop=stop)
            if chain is not None:
                tc.chain_iter_dep(chain, inst.ins)
            return inst

        def chain_dve(inst):
            tc.chain_iter_dep("dve_norm", inst.ins)
            return inst

        # ---- constants set up on-engine (no DMA) --------------------------
        Vt = [const.tile([P, NCH * 65], BF16, name=f"Vt{i}") for i in range(2)]
        for i in range(2):
            nc.gpsimd.memset(
                Vt[i].rearrange("p (c k) -> p c k", k=65)[:, :, 64], 1.0
            )

        # ---- kv-side inputs on the gpsimd queue ---------------------------
        wk_sb = const.tile([P, 8, 128], BF16)
        nc.gpsimd.dma_start(out=wk_sb, in_=wk)
        wv_sb = const.tile([P, 8, 128], BF16)
        nc.gpsimd.dma_start(out=wv_sb, in_=wv)
        cosK_sb = const.tile([P, tkv_c], BF16)
        nc.gpsimd.dma_start(out=cosK_sb, in_=cosK)
        sinK_sb = const.tile([P, tkv_c], BF16)
        nc.gpsimd.dma_start(out=sinK_sb, in_=sinK)
        mask_sb = const.tile([P, NCH], F32)
        nc.gpsimd.dma_start(out=mask_sb, in_=maskb)
        kv_sb = [const.tile([P, 4, tkv_c], BF16, name=f"kv{h}") for h in range(2)]
        nc.gpsimd.dma_start(out=kv_sb[0], in_=kv_a)
        nc.gpsimd.dma_start(out=kv_sb[1], in_=kv_b)

        # ---- q-side inputs on the scalar queue ----------------------------
        e2_sb = const.tile([64, P], R32)
        nc.scalar.dma_start(out=e2_sb, in_=e2)
        # inv broadcast staging: head0 inv in row 0, head1 inv in row 32,
        # all other rows memset to a safe finite value (multiplied by e2=0).
        invp_tiles = [const.tile([64, t2], R32, name=f"invp{i}") for i in range(3)]
        for tl in invp_tiles:
            nc.gpsimd.memset(tl.bitcast(F32), 1.0)
        wq_sb = const.tile([P, 8, 512], BF16)
        nc.scalar.dma_start(out=wq_sb, in_=wq)
        q_sb = [const.tile([P, 8, t2], BF16, name=f"qsb{i}") for i in range(NT2)]
        nc.scalar.dma_start(out=q_sb[0], in_=q_in[0])
        cosQ_sb = const.tile([P, tq], BF16)
        nc.scalar.dma_start(out=cosQ_sb, in_=cosQ)
        sinQ_sb = const.tile([P, tq], BF16)
        nc.scalar.dma_start(out=sinQ_sb, in_=sinQ)
        wout_sb = const.tile([P, 4, D_MODEL], BF16)
        nc.scalar.dma_start(out=wout_sb, in_=wout)
        for i in range(1, NT2):
            nc.scalar.dma_start(out=q_sb[i], in_=q_in[i])

        Kt = const.tile([P, tkv_c], BF16)

        def kv_slice(d, cols):
            return kv_sb[d // 4][:, d % 4, cols]

        def rope_apply(dest, ps, col0, width, cos_sb, sin_sb):
            """dest[128, width] (SBUF) = rope(ps[128, width] PSUM), positions
            col0..col0+width. Rows are two stacked heads, each [x1(32); x2(32)]."""
            cs = cos_sb[:, col0 : col0 + width]
            t_cos = ropep.tile([P, t2], F32, tag="rope", name="t_cos")
            t_u = ropep.tile([P, t2], F32, tag="rope", name="t_u")
            tc_ = t_cos[:, :width]
            tu_ = t_u[:, :width]
            nc.vector.tensor_mul(tc_, ps, cs)
            for b0 in (0, 64):
                # sin rows [b0:b0+32] = -sin, [b0+32:b0+64] = +sin
                nc.vector.tensor_mul(
                    tu_[b0 : b0 + 32, :],
                    ps[b0 + 32 : b0 + 64, :],
                    sin_sb[b0 : b0 + 32, col0 : col0 + width],
                )
                nc.vector.tensor_mul(
                    tu_[b0 + 32 : b0 + 64, :],
                    ps[b0 : b0 + 32, :],
                    sin_sb[b0 + 32 : b0 + 64, col0 : col0 + width],
                )
            with nc.allow_low_precision("rope output bf16"):
                nc.vector.tensor_add(dest, tc_, tu_)

        # ---- phase KV: K/V projections ------------------------------------
        for kc0, kw in ktiles:
            cols = slice(kc0, kc0 + kw)
            ps_k = pp_big.tile([P, 512], F32, tag="big", name="ps_k")
            pk = ps_k[:, :kw]
            for d in range(8):
                MM(pk, wk_sb[:, d, :], kv_slice(d, cols), d == 0, d == 7)
            rope_apply(Kt[:, cols], pk, kc0, kw, cosK_sb, sinK_sb)
            for s in range(kw // 128):
                ps_v = pp_big.tile([P, 512], F32, tag="big", name="ps_v")
                pv = ps_v[:, 0:128]
                vcols = slice(kc0 + s * 128, kc0 + (s + 1) * 128)
                for d in range(8):
                    MM(pv, kv_slice(d, vcols), wv_sb[:, d, :], d == 0, d == 7)
                c = kc0 // 128 + s
                with nc.allow_low_precision("V bf16"):
                    nc.vector.tensor_copy(
                        out=Vt[0][:, c * 65 : c * 65 + 64], in_=pv[:, 0:64]
                    )
                    nc.vector.tensor_copy(
                        out=Vt[1][:, c * 65 : c * 65 + 64], in_=pv[:, 64:128]
                    )

        # ---- double-generation Qt / attnT tiles ---------------------------
        Qt = [
            [
                qpool.tile([P, t2], BF16, tag=f"Q{j}g{ggen}", name=f"Qt{j}g{ggen}")
                for j in range(NPAIR)
            ]
            for ggen in range(2)
        ]
        At = [
            [
                apool.tile([P, t2], BF16, tag=f"A{j}g{ggen}", name=f"At{j}g{ggen}")
                for j in range(NPAIR)
            ]
            for ggen in range(2)
        ]
        pending = []
        pair_seq = [0]

        def qproj_pair(it2, j):
            ps_q = pp_big.tile([P, t2], F32, tag="big", name="ps_q")
            for half in range(NHALF):
                for d in range(8):
                    MM(
                        ps_q[:, half * 512 : (half + 1) * 512],
                        wq_sb[:, d, j * 128 : (j + 1) * 128],
                        q_sb[it2][:, d, half * 512 : (half + 1) * 512],
                        d == 0,
                        d == 7,
                    )
            rope_apply(Qt[it2 % 2][j], ps_q, it2 * t2, t2, cosQ_sb, sinQ_sb)

        def flush_norm():
            if not pending:
                return
            U0, U1, invp, j_, attn_cur = pending.pop(0)
            Us = (U0, U1)
            for half in range(NHALF):
                hs = slice(half * 512, (half + 1) * 512)
                ps_b = pp_big.tile([P, 512], F32, tag="big", name="ps_b")
                MM(ps_b, e2_sb, invp[:, hs], True, True, chain="pe_attn")
                for ab, base in ((0, 0), (1, 64)):
                    with nc.allow_low_precision("attnT bf16"):
                        chain_dve(
                            nc.vector.tensor_mul(
                                attn_cur[j_][base : base + 64, hs],
                                Us[ab][0:64, hs],
                                ps_b[base : base + 64, :],
                            )
                        )

        def outproj_slices(it2, slices):
            attn_cur = At[it2 % 2]
            for s in slices:
                ob = ropep.tile([P, D_MODEL], F32, tag="ob", name="ob", bufs=2)
                for n in range(2):
                    ps_f = pp_big.tile([P, 512], F32, tag="big", name="ps_f")
                    for p_ in range(NPAIR):
                        MM(
                            ps_f,
                            attn_cur[p_][:, s * 128 : (s + 1) * 128],
                            wout_sb[:, p_, n * 512 : (n + 1) * 512],
                            p_ == 0,
                            p_ == NPAIR - 1,
                        )
                    nc.scalar.copy(out=ob[:, n * 512 : (n + 1) * 512], in_=ps_f)
                r0 = it2 * t2 + s * 128
                nc.sync.dma_start(out=out[r0 : r0 + 128, :], in_=ob)

        def attn_block(it2):
            Qt_cur = Qt[it2 % 2]
            attn_cur = At[it2 % 2]
            nsl = 0  # outproj slices of the previous block already emitted
            for j in range(NPAIR):
                heads = [(j, 0, 0), (j + 4, 1, 64)]  # (head, kvh, base)
                ps_os = [
                    pp_acc.tile([65, t2], F32, tag="acc", name=f"ps_o{ab}")
                    for ab in range(2)
                ]

                def emit_pv(c_, exs_):
                    for ab, (_h, kvh, _base) in enumerate(heads):
                        for half in range(NHALF):
                            MM(
                                ps_os[ab][:, half * 512 : (half + 1) * 512],
                                Vt[kvh][:, c_ * 65 : c_ * 65 + 65],
                                exs_[ab][:, half * 512 : (half + 1) * 512],
                                c_ == 0,
                                c_ == NCH - 1,
                                chain="pe_attn",
                            )

                # PV lags the scores by one chunk so no PE instruction ever
                # reaches the queue head with an unresolved wait.
                prev = None
                for c in range(NCH):
                    exs = []
                    for ab, (_h, kvh, base) in enumerate(heads):
                        ps_s = pp_big.tile([P, t2], F32, tag="big", name="ps_s")
                        for half in range(NHALF):
                            MM(
                                ps_s[:, half * 512 : (half + 1) * 512],
                                Kt[base : base + 64, c * 128 : (c + 1) * 128],
                                Qt_cur[j][base : base + 64, half * 512 : (half + 1) * 512],
                                True,
                                True,
                                chain="pe_attn",
                            )
                        ex = workp.tile([P, t2], BF16, tag="expT", name="ex", bufs=6)
                        nc.scalar.activation(
                            out=ex,
                            in_=ps_s,
                            func=mybir.ActivationFunctionType.Exp,
                            bias=mask_sb[:, c : c + 1],
                            scale=0.125,
                        )
                        exs.append(ex)
                    if prev is not None:
                        emit_pv(c - 1, prev)
                    prev = exs
                emit_pv(NCH - 1, prev)
                # flush the previous pair first: its bcast matmul runs now
                # (reciprocal long done), and its muls free ps_b slots early.
                while pending:
                    flush_norm()
                # accumulator copies BEFORE the reciprocal: the in-order DVE
                # must release both PSUM slots promptly.
                invp = invp_tiles[pair_seq[0] % 3]
                pair_seq[0] += 1
                Us = []
                for ab in range(2):
                    U = workp.tile([64, t2], F32, tag="unorm", name="U", bufs=4)
                    chain_dve(nc.vector.tensor_copy(out=U, in_=ps_os[ab][0:64, :]))
                    den = workp.tile([1, t2], F32, tag="den", name="den", bufs=3)
                    chain_dve(
                        nc.vector.tensor_copy(out=den, in_=ps_os[ab][64:65, :])
                    )
                    inv_f = workp.tile([1, t2], F32, tag="invf", name="inv_f", bufs=3)
                    chain_dve(nc.vector.reciprocal_approx_fast(out=inv_f, in_=den))
                    with nc.allow_low_precision("f32r softmax denom"):
                        chain_dve(
                            nc.vector.tensor_copy(
                                out=invp[32 * ab : 32 * ab + 1, :], in_=inv_f
                            )
                        )
                    Us.append(U)
                pending.append((Us[0], Us[1], invp, j, attn_cur))

                # interleaved cross-block work
                if it2 + 1 < NT2:
                    qproj_pair(it2 + 1, j)
                if it2 > 0:
                    take = NSLICE * (j + 1) // NPAIR
                    outproj_slices(it2 - 1, range(nsl, take))
                    nsl = take

        # ---- pipeline -----------------------------------------------------
        for j in range(NPAIR):
            qproj_pair(0, j)
        for it2 in range(NT2):
            attn_block(it2)
        while pending:
            flush_norm()
        outproj_slices(NT2 - 1, range(NSLICE))

    nc.compile()
    return nc


# ---------------------------------------------------------------------------
# host-side sharding / prep
# ---------------------------------------------------------------------------

_HEAD_PERM = [0, 4, 1, 5, 2, 6, 3, 7]  # local head order inside pair tiles


def _rope_tables(positions):
    """cos/sin tables [128, len(positions)] with the sign pattern baked in."""
    theta = ROPE_BASE ** (-np.arange(0, D_K, 2, dtype=np.float64) / D_K)  # [32]
    ang = positions.astype(np.float64)[:, None] * theta[None, :]  # [T,32]
    c = np.cos(ang).T.astype(np.float32)  # [32, T]
    s = np.sin(ang).T.astype(np.float32)
    cosF = np.concatenate([c, c, c, c], axis=0)
    sinF = np.concatenate([-s, s, -s, s], axis=0)
    return np.ascontiguousarray(cosF), np.ascontiguousarray(sinF)


def _pack8(a, c, width):
    """[c*128, width] -> [128, c, width] with row d = c_idx*128 + p."""
    return np.ascontiguousarray(a.reshape(c, 128, width).transpose(1, 0, 2))


def make_in_maps(query, key_value, kv_mask, w_q, w_k, w_v, w_out, tq=TQ):
    nb = query.shape[0]
    bf = ml_dtypes.bfloat16

    idxs = [np.nonzero(kv_mask[b])[0] for b in range(nb)]
    nmax = max((len(i) for i in idxs), default=1)
    tkv_c = max(256, int(math.ceil(max(nmax, 1) / 128.0)) * 128)
    nch = tkv_c // 128

    cosQ, sinQ = _rope_tables(np.arange(tq))
    cosQ_bf = cosQ.astype(bf)
    sinQ_bf = sinQ.astype(bf)
    e2 = np.zeros((64, 128), np.float32)
    e2[0, 0:64] = 1.0
    e2[32, 64:128] = 1.0

    col_perm = np.concatenate(
        [np.arange(h * D_K, (h + 1) * D_K) for h in _HEAD_PERM]
    )
    in_maps = []
    for core in range(2 * nb):
        b = core // 2
        g = core % 2
        idx = idxs[b]
        nv = len(idx)

        kv_c = np.zeros((tkv_c, D_MODEL), np.float32)
        kv_c[:nv] = key_value[b][idx]
        kvT = np.ascontiguousarray(kv_c.T)  # [1024, tkv_c]

        pos = np.zeros(tkv_c, np.int64)
        pos[:nv] = idx
        cosK, sinK = _rope_tables(pos)

        maskb = np.full(tkv_c, NEG_BIAS, np.float32)
        maskb[:nv] = 0.0
        maskb = np.ascontiguousarray(maskb.reshape(nch, 128).T)

        qT = np.ascontiguousarray(query[b].T)  # [1024, tq]

        wq_g = w_q[:, g * 512 : (g + 1) * 512][:, col_perm]
        wk_g = w_k[:, g * 128 : (g + 1) * 128]
        wv_g = w_v[:, g * 128 : (g + 1) * 128]
        wout_g = w_out[g * 512 : (g + 1) * 512, :][col_perm, :]

        m = {
            "kv_a": _pack8(kvT[0:512], 4, tkv_c).astype(bf),
            "kv_b": _pack8(kvT[512:1024], 4, tkv_c).astype(bf),
            "wq": _pack8(np.ascontiguousarray(wq_g), 8, 512).astype(bf),
            "wk": _pack8(np.ascontiguousarray(wk_g), 8, 128).astype(bf),
            "wv": _pack8(np.ascontiguousarray(wv_g), 8, 128).astype(bf),
            "wout": _pack8(np.ascontiguousarray(wout_g), 4, D_MODEL).astype(bf),
            "cosK": cosK.astype(bf),
            "sinK": sinK.astype(bf),
            "cosQ": cosQ_bf,
            "sinQ": sinQ_bf,
            "maskb": maskb,
            "e2": e2,
        }
        for i in range(tq // 1024):
            m[f"q{i}"] = _pack8(
                np.ascontiguousarray(qT[:, i * 1024 : (i + 1) * 1024]), 8, 1024
            ).astype(bf)
        in_maps.append(m)
    return in_maps, tkv_c


_NC_CACHE = {}


def _get_nc(tq, tkv_c):
    key = (tq, tkv_c)
    if key not in _NC_CACHE:
        _NC_CACHE[key] = build_bass(tq, tkv_c)
    return _NC_CACHE[key]


def _run(inputs, trace=False):
    query = np.asarray(inputs["query"], dtype=np.float32)
    key_value = np.asarray(inputs["key_value"], dtype=np.float32)
    kv_mask = np.asarray(inputs["kv_mask"])
    w_q = np.asarray(inputs["w_q"], dtype=np.float32)
    w_k = np.asarray(inputs["w_k"], dtype=np.float32)
    w_v = np.asarray(inputs["w_v"], dtype=np.float32)
    w_out = np.asarray(inputs["w_out"], dtype=np.float32)
    nb, tq, _ = query.shape

    in_maps, tkv_c = make_in_maps(query, key_value, kv_mask, w_q, w_k, w_v, w_out, tq)
    nc = _get_nc(tq, tkv_c)
    res = run_bass_kernel_spmd(
        nc, in_maps, list(range(2 * nb)), trace=trace, trace_cores=[0]
    )
    outs = [np.asarray(r["out"]) for r in res.results]
    full = np.stack([outs[2 * b] + outs[2 * b + 1] for b in range(nb)])

    query_mask = np.asarray(inputs["query_mask"])
    if not query_mask.all():
        # masked query rows: reference yields uniform attention over all kv
        for b in range(nb):
            rows = ~query_mask[b]
            if rows.any():
                V = key_value[b] @ w_v  # [tkv, 256]
                meanV = V.mean(axis=0)  # [256]
                group = N_HEADS // NUM_KV_HEADS
                feat = np.concatenate([meanV.reshape(NUM_KV_HEADS, D_K)[h // group]
                                       for h in range(N_HEADS)])
                full[b, rows, :] = feat @ w_out
    return full.astype(np.float32), res


def kernel(**inputs):
    out, _ = _run(inputs, trace=False)
    return out


def kernel_traced(**inputs):
    out, res = _run(inputs, trace=True)
    return out, res


if __name__ == "__main__":
    print("kernel.py is a library; use test.py")
